# revision 14
# baseline (speedup 1.0000x reference)
"""Bahdanau attention kernel for 8 Trainium2 NeuronCores.

Problem shapes (hardcoded): hidden [2, 32, 1024], encoder_outputs [32, 2048, 1024],
Wq/Wk [1024, 1024], bq/bk/wv [1024], bv scalar. Output [32, 1, 1024].

Sharding: data-parallel over batch B=32 -> 4 batches per core, weights replicated.
bv is dropped entirely (softmax is invariant to constant shifts).

Key structure (v8):
- The PE's main job is the K-projection (enc @ Wk.T) in fp8e4 DoubleRow
  (2 fp8 MACs/cell/cycle): 4 accumulating MMs of contraction 256 per
  (o-tile, s-chunk). Wk is pre-scaled x64 on the host; the inverse folds
  into the tanh scale. The q-projection also runs fp8 DR, interleaved into
  chunk (0,0)'s chains so it costs no serial ramp time.
- The tanh output is written in fp8e4 (its ONLY consumer is the scores
  contraction; the einsum reads the bf16 enc copy directly), so the scores
  wv-contraction also runs in fp8 DoubleRow: 4 chained MMs per chunk with
  RANK-1 paired weights (wvmat8[:, u, r, m] = 64*wv[128(2u+r)+p]), each
  contracting 2 o-chunks x 128 partitions at once. The [128, 512] PSUM
  result is 64*scores replicated on every partition - the einsum broadcast
  comes for free and the x64 folds into the exp's free scale. No transposes
  and no cross-engine scores traffic anywhere.
- exp on ACT straight from the scores PSUM with scale=1/64 (no max-shift
  needed: |scores| <= sum|wv| <= 32), accum_out -> softmax denominator.
- attn @ enc einsum: 8 scalar_tensor_tensor ops with accum_out on DVE over
  the transposed bf16 enc tiles (same [h-part, s] tiling as the fp8 tiles);
  per-chunk partials land in acc4[:, i, j]; one reduce at batch end.
- The final [128, 8] column is scaled by the reciprocal softmax sum and
  written transposed (outT); the host undoes the transpose.
- Ramp: the two DMA queues run in parallel (scalar: first fp8 enc chunk +
  per-tile wqT8 + smalls; sync: wkT8 halves + the enc rings).
"""

from contextlib import ExitStack

import numpy as np

import concourse.bacc as bacc
import concourse.bass as bass
import concourse.mybir as mybir
import concourse.tile as tile
from concourse.bass_utils import run_bass_kernel_spmd

B, S, H = 32, 2048, 1024
NCORES = 8
BPC = B // NCORES  # 4 batches per core
F32 = mybir.dt.float32
BF16 = mybir.dt.bfloat16
FP8 = mybir.dt.float8e4
HT = H // 128  # 8 chunks of 128 along h or o
HU = HT // 2  # 4 DoubleRow o-chunk pairs for the scores contraction
SC = S // 512  # 4 s-chunks of 512
KT = 4  # fp8 DoubleRow: 4 contraction steps of 256
WK_SCALE = 64.0
HID_SCALE = 8.0
Tanh = mybir.ActivationFunctionType.Tanh
Exp = mybir.ActivationFunctionType.Exp
X = mybir.AxisListType.X
DR = mybir.MatmulPerfMode.DoubleRow
Mult = mybir.AluOpType.mult
Add = mybir.AluOpType.add

ts = bass.ts


def build_program():
    nc = bacc.Bacc("TRN2", target_bir_lowering=False, debug=False)

    # enc^T fp8 tiles: encT8[b, j, p, i, s] = fp8(enc[b, 512j+s, 128i+p])
    encT8_d = nc.dram_tensor("encT8", [BPC, SC, 128, HT, 512], FP8, kind="ExternalInput")
    # enc^T bf16 tiles, same layout (einsum operand)
    encT16_d = nc.dram_tensor("encT16", [BPC, SC, 128, HT, 512], BF16, kind="ExternalInput")
    # Wk^T fp8 (x64): wkT8[p, i, c, m] = fp8(64 * Wk[128i+m, 128c+p])
    wkT8_d = nc.dram_tensor("wkT8", [128, HT, HT, 128], FP8, kind="ExternalInput")
    # Wq^T fp8 (x64): wqT8[p, t, c, n] = fp8(64 * Wq[128t+n, 128c+p])
    wqT8_d = nc.dram_tensor("wqT8", [128, HT, HT, 128], FP8, kind="ExternalInput")
    # hid^T fp8 (x8): hidT8[p, c, b] = fp8(8 * hidden[-1][b, 128c+p])
    hidT8_d = nc.dram_tensor("hidT8", [128, HT, BPC], FP8, kind="ExternalInput")
    bqkT_d = nc.dram_tensor("bqkT", [128, HT], F32, kind="ExternalInput")  # (bq+bk)^T
    # rank-1 paired scores weights (x64): wvmat8[p, u, r, m] = fp8(64*wv[128(2u+r)+p])
    wvmat8_d = nc.dram_tensor("wvmat8", [128, HU, 2, 128], FP8, kind="ExternalInput")
    # transposed output: outT[b, p, i] = out[b, 128i+p]
    outT_d = nc.dram_tensor("outT", [BPC, 128, HT], F32, kind="ExternalOutput")

    with tile.TileContext(nc) as tc, ExitStack() as ctx:
        consts = ctx.enter_context(tc.tile_pool(name="consts", bufs=1))
        kp = ctx.enter_context(tc.tile_pool(name="kp", bufs=5, space="PSUM"))
        ps_p = ctx.enter_context(tc.tile_pool(name="psp", bufs=2, space="PSUM"))
        pq_p = ctx.enter_context(tc.tile_pool(name="pqp", bufs=1, space="PSUM"))
        encT_p = ctx.enter_context(tc.tile_pool(name="encT", bufs=5))  # 512KB/slot
        enc16_p = ctx.enter_context(tc.tile_pool(name="enc16", bufs=6))  # 1MB/slot
        eT_p = ctx.enter_context(tc.tile_pool(name="eT", bufs=2))  # 512KB/slot
        sc_p = ctx.enter_context(tc.tile_pool(name="sc", bufs=2))
        batch = ctx.enter_context(tc.tile_pool(name="batch", bufs=1))

        # ---- staging helpers (chunk granular DMAs; the fp8 kproj operand
        # rides the sync queue, the bf16 einsum operand rides the otherwise
        # idle gpsimd queue so neither stream blocks the other) ----
        def load_enc_chunk(b, j):
            eT8 = encT_p.tile([128, HT, 512], FP8, tag="encT8")
            nc.sync.dma_start(eT8[:], encT8_d[b, j])
            e16 = enc16_p.tile([128, HT, 512], BF16, tag="enc16")
            nc.gpsimd.dma_start(e16[:], encT16_d[b, j])
            return eT8, e16

        # ---- weights + consts. Critical path to the first kproj chain:
        # scalar queue carries the first fp8 enc chunk; sync queue carries
        # wkT8's lower half. Everything else lands behind those. ----
        wkT8 = consts.tile([128, HT, HT, 128], FP8, tag="wkT8")
        eT8_00 = encT_p.tile([128, HT, 512], FP8, tag="encT8", name="eT8_00")
        hidT8 = consts.tile([128, HT, BPC], FP8, tag="hidT8")
        bqkT = consts.tile([128, HT], F32, tag="bqkT")
        wqT8 = consts.tile([128, HT, HT, 128], FP8, tag="wqT8")
        wvmat8 = consts.tile([128, HU, 2, 128], FP8, tag="wvmat8")

        # Ramp: spread the critical first transfers across all three DMA
        # queues (each dispatch costs ~0.65us on its queue, so only the
        # first-needed items are fine-grained). kproj chain 0 needs
        # wkT8[:,0] + eT8_00; later chains' weights ride the gpsimd queue.
        nc.sync.dma_start(wkT8[:, 0], wkT8_d[:, 0])
        nc.sync.dma_start(eT8_00[:, 0:4, :], encT8_d[0, 0, :, 0:4, :])
        nc.sync.dma_start(eT8_00[:, 4:8, :], encT8_d[0, 0, :, 4:8, :])
        nc.gpsimd.dma_start(wkT8[:, 1:4], wkT8_d[:, 1:4])
        nc.gpsimd.dma_start(wkT8[:, 4:8], wkT8_d[:, 4:8])
        nc.scalar.dma_start(hidT8[:], hidT8_d[:])
        nc.scalar.dma_start(bqkT[:], bqkT_d[:])
        nc.scalar.dma_start(wvmat8[:], wvmat8_d[:])
        for t in range(HT):
            nc.scalar.dma_start(wqT8[:, t], wqT8_d[:, t])

        eT8_01 = encT_p.tile([128, HT, 512], FP8, tag="encT8", name="eT8_01")
        nc.sync.dma_start(eT8_01[:], encT8_d[0, 1])
        e16_00 = enc16_p.tile([128, HT, 512], BF16, tag="enc16", name="e16_00")
        nc.gpsimd.dma_start(e16_00[:], encT16_d[0, 0])
        e16_01 = enc16_p.tile([128, HT, 512], BF16, tag="enc16", name="e16_01")
        nc.gpsimd.dma_start(e16_01[:], encT16_d[0, 1])
        staged = {(0, 0): (eT8_00, e16_00), (0, 1): (eT8_01, e16_01)}
        staged[(0, 2)] = load_enc_chunk(0, 2)
        staged[(0, 3)] = load_enc_chunk(0, 3)

        qkb = consts.tile([128, HT, BPC], F32, tag="qkb")
        scr = consts.tile([128, 512], BF16, tag="scr")

        def kproj_mm_chain(i, eT8):
            pk = kp.tile([128, 512], F32, tag="kp", name="pk")
            for kt in range(KT):
                nc.tensor.matmul(
                    pk[:],
                    wkT8[:, i, ts(kt, 2), :],
                    eT8[:, ts(kt, 2), :],
                    start=(kt == 0),
                    stop=(kt == KT - 1),
                    perf_mode=DR,
                )
            return pk

        def qproj_t(t):
            # q^T + bq + bk, one o-tile: qkb[:, t, b] = q/512 + bq + bk
            pq = pq_p.tile([128, BPC], F32, tag="pq")
            for kt in range(KT):
                nc.tensor.matmul(
                    pq[:],
                    wqT8[:, t, ts(kt, 2), :],
                    hidT8[:, ts(kt, 2), :],
                    start=(kt == 0),
                    stop=(kt == KT - 1),
                    perf_mode=DR,
                )
            nc.vector.tensor_scalar(
                qkb[:, t, :], pq[:], 1.0 / (WK_SCALE * HID_SCALE),
                bqkT[:, t : t + 1], op0=Mult, op1=Add,
            )

        def scores_mm(c, u):
            # 64*scores chunk via fp8 DR: contraction over 2 o-chunks x 128
            # partitions at once; M=128 replicated rows = broadcast result.
            nc.tensor.matmul(
                c["ps"][:],
                wvmat8[:, u],
                c["eT_j"][:, ts(u, 2), :],
                start=(u == 0),
                stop=(u == HU - 1),
                perf_mode=DR,
            )

        def tail_exp(p):
            attn = sc_p.tile([128, 512], BF16, tag="attn")
            nc.scalar.activation(
                attn[:], p["ps"][:], Exp, scale=1.0 / WK_SCALE,
                accum_out=ssum4s[p["b"]][:, p["j"] : p["j"] + 1],
            )
            p["attn"] = attn

        def tail_einsum(p):
            # attn-weighted sum of enc rows on DVE: acc4[p_, i, j] =
            # sum_s attn[s] * enc[512j+s, 128i+p_].
            b, j = p["b"], p["j"]
            for i in range(HT):
                nc.vector.scalar_tensor_tensor(
                    scr[:], p["e16"][:, i, :], 1.0, p["attn"][:],
                    op0=Mult, op1=Mult,
                    accum_out=acc4s[b][:, i, j : j + 1],
                )

        def finish_batch(b):
            accH = batch.tile([128, HT], F32, tag="accH", bufs=2)
            nc.vector.reduce_sum(accH[:], acc4s[b][:], axis=X)
            ssum = batch.tile([128, 1], F32, tag="ssum", bufs=2)
            nc.vector.reduce_sum(ssum[:], ssum4s[b][:], axis=X)
            inv = batch.tile([128, 1], F32, tag="inv", bufs=2)
            nc.vector.reciprocal(inv[:], ssum[:])
            outF = batch.tile([128, HT], F32, tag="outF", bufs=2)
            nc.vector.tensor_scalar_mul(outF[:], accH[:], inv[:, 0:1])
            nc.gpsimd.dma_start(outT_d[b][:], outF[:])

        # per-batch accumulators
        acc4s = [
            batch.tile([128, HT, SC], F32, tag=f"acc4_{b}", name=f"acc4_{b}")
            for b in range(BPC)
        ]
        ssum4s = [
            batch.tile([128, SC], F32, tag=f"ssum4_{b}", name=f"ssum4_{b}")
            for b in range(BPC)
        ]

        pending = None
        for b in range(BPC):
            for j in range(SC):
                # stage chunk (b+1, j) while computing chunk (b, j)
                if b + 1 < BPC:
                    staged[(b + 1, j)] = load_enc_chunk(b + 1, j)
                eT8, e16 = staged.pop((b, j))
                p = pending
                first = b == 0 and j == 0
                cur = dict(
                    b=b, j=j, e16=e16,
                    eT_j=eT_p.tile([128, HT, 512], FP8, tag="eTj", name="eTj"),
                    ps=ps_p.tile([128, 512], F32, tag="ps", name="ps"),
                )
                for i in range(HT):
                    pk = kproj_mm_chain(i, eT8)
                    if first:
                        qproj_t(i)
                    nc.scalar.activation(
                        cur["eT_j"][:, i, :], pk[:], Tanh,
                        bias=qkb[:, i, b : b + 1], scale=1.0 / WK_SCALE,
                    )
                    if i == 0 and p is not None:
                        # prev chunk's last scores MM + its tail
                        scores_mm(p, HU - 1)
                        tail_exp(p)
                        tail_einsum(p)
                        if p["j"] == SC - 1:
                            finish_batch(p["b"])
                    if i >= 3 and i % 2 == 1:
                        # scores DR pair (i-3, i-2) of THIS chunk: both tanhs
                        # are done by the time chain_i finishes streaming
                        scores_mm(cur, (i - 3) // 2)
                pending = cur

        # flush the final chunk
        p = pending
        scores_mm(p, HU - 1)
        tail_exp(p)
        tail_einsum(p)
        finish_batch(p["b"])

    nc.compile()
    return nc


_CACHED_NC = None


def _get_nc():
    global _CACHED_NC
    if _CACHED_NC is None:
        _CACHED_NC = build_program()
    return _CACHED_NC


_F8 = mybir.dt.np(FP8)
_BF = mybir.dt.np(BF16)


def make_in_maps(hidden, encoder_outputs, Wq, bq, Wk, bk, wv):
    """Host-side shard + layout prep (all compute FLOPs stay on device)."""
    hid_last = np.asarray(hidden, np.float32)[-1]  # [32, H]
    enc = np.asarray(encoder_outputs, np.float32)
    Wq = np.asarray(Wq, np.float32)
    Wk = np.asarray(Wk, np.float32)
    wv = np.asarray(wv, np.float32)
    bqkT = np.ascontiguousarray(
        (np.asarray(bq, np.float32) + np.asarray(bk, np.float32)).reshape(HT, 128).T
    )
    # rank-1 paired scores weights: [128(p), HU(u), 2(r), 128(m)] = 64*wv
    wvT = wv.reshape(HT, 128).T  # [128, HT]
    wvmat8 = np.ascontiguousarray(
        np.broadcast_to(
            (wvT * WK_SCALE).reshape(128, HU, 2)[:, :, :, None], (128, HU, 2, 128)
        )
    ).astype(_F8)

    # enc^T tiles: [B, SC, 128(p), HT(i), 512(s)]
    encT = np.ascontiguousarray(
        np.clip(enc, -240, 240)
        .reshape(B, SC, 512, HT, 128)
        .transpose(0, 1, 4, 3, 2)
    )
    encT8 = encT.astype(_F8)
    encT16 = encT.astype(_BF)

    # W^T fp8 with x64 prescale: [128(p), HT(i), HT(c), 128(m)]
    def wT8(W):
        return np.ascontiguousarray(
            np.clip(W * WK_SCALE, -240, 240)
            .reshape(HT, 128, HT, 128)
            .transpose(3, 0, 2, 1)
        ).astype(_F8)

    wkT8 = wT8(Wk)
    wqT8 = wT8(Wq)

    in_maps = []
    for c in range(NCORES):
        sl = slice(c * BPC, (c + 1) * BPC)
        # hid^T fp8 x8: [128(p), HT(c), BPC(b)]
        hidT8 = np.ascontiguousarray(
            np.clip(hid_last[sl] * HID_SCALE, -240, 240)
            .reshape(BPC, HT, 128)
            .transpose(2, 1, 0)
        ).astype(_F8)
        in_maps.append(
            {
                "encT8": np.ascontiguousarray(encT8[sl]),
                "encT16": np.ascontiguousarray(encT16[sl]),
                "wkT8": wkT8,
                "wqT8": wqT8,
                "hidT8": hidT8,
                "bqkT": bqkT,
                "wvmat8": wvmat8,
            }
        )
    return in_maps


def run(inputs, trace=False):
    """Run on hardware; returns (output [32,1,1024], BassKernelResults)."""
    nc = _get_nc()
    in_maps = make_in_maps(
        inputs["hidden"],
        inputs["encoder_outputs"],
        inputs["Wq"],
        inputs["bq"],
        inputs["Wk"],
        inputs["bk"],
        inputs["wv"],
    )
    res = run_bass_kernel_spmd(nc, in_maps, list(range(NCORES)), trace=trace)
    # outT[b, p, i] -> out[b, 128i+p]
    outs = []
    for c in range(NCORES):
        outT = res.results[c]["outT"]  # [BPC, 128, HT]
        outs.append(outT.transpose(0, 2, 1).reshape(BPC, 1, H))
    out = np.concatenate(outs, axis=0)
    return out.astype(np.float32), res


def kernel(hidden, encoder_outputs, Wq, bq, Wk, bk, wv, bv):
    out, _ = run(
        {
            "hidden": hidden,
            "encoder_outputs": encoder_outputs,
            "Wq": Wq,
            "bq": bq,
            "Wk": Wk,
            "bk": bk,
            "wv": wv,
        }
    )
    return out


# revision 18
# speedup vs baseline: 1.0812x; 1.0812x over previous
"""Bahdanau attention kernel for 8 Trainium2 NeuronCores.

Problem shapes (hardcoded): hidden [2, 32, 1024], encoder_outputs [32, 2048, 1024],
Wq/Wk [1024, 1024], bq/bk/wv [1024], bv scalar. Output [32, 1, 1024].

Sharding: data-parallel over batch B=32 -> 4 batches per core, weights replicated.
bv is dropped entirely (softmax is invariant to constant shifts).

Key structure (v8):
- The PE's main job is the K-projection (enc @ Wk.T) in fp8e4 DoubleRow
  (2 fp8 MACs/cell/cycle): 4 accumulating MMs of contraction 256 per
  (o-tile, s-chunk). Wk is pre-scaled x64 on the host; the inverse folds
  into the tanh scale. The q-projection also runs fp8 DR, interleaved into
  chunk (0,0)'s chains so it costs no serial ramp time.
- The tanh output is written in fp8e4 (its ONLY consumer is the scores
  contraction; the einsum reads the bf16 enc copy directly), so the scores
  wv-contraction also runs in fp8 DoubleRow: 4 chained MMs per chunk with
  RANK-1 paired weights (wvmat8[:, u, r, m] = 64*wv[128(2u+r)+p]), each
  contracting 2 o-chunks x 128 partitions at once. The [128, 512] PSUM
  result is 64*scores replicated on every partition - the einsum broadcast
  comes for free and the x64 folds into the exp's free scale. No transposes
  and no cross-engine scores traffic anywhere.
- exp on ACT straight from the scores PSUM with scale=1/64 (no max-shift
  needed: |scores| <= sum|wv| <= 32), accum_out -> softmax denominator.
- attn @ enc einsum: 8 scalar_tensor_tensor ops with accum_out on DVE over
  the transposed bf16 enc tiles (same [h-part, s] tiling as the fp8 tiles);
  per-chunk partials land in acc4[:, i, j]; one reduce at batch end.
- The final [128, 8] column is scaled by the reciprocal softmax sum and
  written transposed (outT); the host undoes the transpose.
- Ramp: the two DMA queues run in parallel (scalar: first fp8 enc chunk +
  per-tile wqT8 + smalls; sync: wkT8 halves + the enc rings).
"""

from contextlib import ExitStack

import numpy as np

import concourse.bacc as bacc
import concourse.bass as bass
import concourse.mybir as mybir
import concourse.tile as tile
from concourse.bass_utils import run_bass_kernel_spmd

B, S, H = 32, 2048, 1024
NCORES = 8
BPC = B // NCORES  # 4 batches per core
F32 = mybir.dt.float32
BF16 = mybir.dt.bfloat16
FP8 = mybir.dt.float8e4
HT = H // 128  # 8 chunks of 128 along h or o
HU = HT // 2  # 4 DoubleRow o-chunk pairs for the scores contraction
SC = S // 512  # 4 s-chunks of 512
KT = 4  # fp8 DoubleRow: 4 contraction steps of 256
WK_SCALE = 64.0
HID_SCALE = 8.0
Tanh = mybir.ActivationFunctionType.Tanh
Exp = mybir.ActivationFunctionType.Exp
X = mybir.AxisListType.X
DR = mybir.MatmulPerfMode.DoubleRow
Mult = mybir.AluOpType.mult
Add = mybir.AluOpType.add

ts = bass.ts


def build_program():
    nc = bacc.Bacc("TRN2", target_bir_lowering=False, debug=False)

    # enc^T fp8 tiles: encT8[b, j, p, i, s] = fp8(enc[b, 512j+s, 128i+p])
    encT8_d = nc.dram_tensor("encT8", [BPC, SC, 128, HT, 512], FP8, kind="ExternalInput")
    # enc^T bf16 tiles, same layout (einsum operand)
    encT16_d = nc.dram_tensor("encT16", [BPC, SC, 128, HT, 512], BF16, kind="ExternalInput")
    # Wk^T fp8 (x64): wkT8[p, i, c, m] = fp8(64 * Wk[128i+m, 128c+p])
    wkT8_d = nc.dram_tensor("wkT8", [128, HT, HT, 128], FP8, kind="ExternalInput")
    # Wq^T fp8 (x64): wqT8[p, t, c, n] = fp8(64 * Wq[128t+n, 128c+p])
    wqT8_d = nc.dram_tensor("wqT8", [128, HT, HT, 128], FP8, kind="ExternalInput")
    # hid^T fp8 (x8): hidT8[p, c, b] = fp8(8 * hidden[-1][b, 128c+p])
    hidT8_d = nc.dram_tensor("hidT8", [128, HT, BPC], FP8, kind="ExternalInput")
    bqkT_d = nc.dram_tensor("bqkT", [128, HT], F32, kind="ExternalInput")  # (bq+bk)^T
    # rank-1 paired scores weights (x64): wvmat8[p, u, r, m] = fp8(64*wv[128(2u+r)+p])
    wvmat8_d = nc.dram_tensor("wvmat8", [128, HU, 2, 128], FP8, kind="ExternalInput")
    # transposed output: outT[b, p, i] = out[b, 128i+p]
    outT_d = nc.dram_tensor("outT", [BPC, 128, HT], F32, kind="ExternalOutput")

    with tile.TileContext(nc) as tc, ExitStack() as ctx:
        consts = ctx.enter_context(tc.tile_pool(name="consts", bufs=1))
        kp = ctx.enter_context(tc.tile_pool(name="kp", bufs=5, space="PSUM"))
        ps_p = ctx.enter_context(tc.tile_pool(name="psp", bufs=2, space="PSUM"))
        pq_p = ctx.enter_context(tc.tile_pool(name="pqp", bufs=1, space="PSUM"))
        encT_p = ctx.enter_context(tc.tile_pool(name="encT", bufs=5))  # 512KB/slot
        enc16_p = ctx.enter_context(tc.tile_pool(name="enc16", bufs=6))  # 1MB/slot
        eT_p = ctx.enter_context(tc.tile_pool(name="eT", bufs=2))  # 512KB/slot
        sc_p = ctx.enter_context(tc.tile_pool(name="sc", bufs=2))
        batch = ctx.enter_context(tc.tile_pool(name="batch", bufs=1))

        # ---- staging helpers (chunk granular DMAs; the fp8 kproj operand
        # rides the sync queue, the bf16 einsum operand rides the otherwise
        # idle gpsimd queue so neither stream blocks the other) ----
        def load_enc_chunk(b, j):
            eT8 = encT_p.tile([128, HT, 512], FP8, tag="encT8")
            nc.sync.dma_start(eT8[:], encT8_d[b, j])
            e16 = enc16_p.tile([128, HT, 512], BF16, tag="enc16")
            nc.sync.dma_start(e16[:], encT16_d[b, j])
            return eT8, e16

        # ---- weights + consts. Critical path to the first kproj chain:
        # scalar queue carries the first fp8 enc chunk; sync queue carries
        # wkT8's lower half. Everything else lands behind those. ----
        wkT8 = consts.tile([128, HT, HT, 128], FP8, tag="wkT8")
        eT8_00 = encT_p.tile([128, HT, 512], FP8, tag="encT8", name="eT8_00")
        hidT8 = consts.tile([128, HT, BPC], FP8, tag="hidT8")
        bqkT = consts.tile([128, HT], F32, tag="bqkT")
        wqT8 = consts.tile([128, HT, HT, 128], FP8, tag="wqT8")
        wvmat8 = consts.tile([128, HU, 2, 128], FP8, tag="wvmat8")

        # Ramp: scalar queue carries the first fp8 enc chunk + qproj inputs;
        # the sync queue carries wkT8 halves then batch 0's chunks strictly
        # in need-time order (the bf16 einsum copies are consumed one chunk
        # later than the fp8 tiles, so they interleave behind them).
        nc.sync.dma_start(wkT8[:, 0:4], wkT8_d[:, 0:4])
        nc.scalar.dma_start(eT8_00[:], encT8_d[0, 0])
        nc.sync.dma_start(wkT8[:, 4:8], wkT8_d[:, 4:8])
        nc.scalar.dma_start(hidT8[:], hidT8_d[:])
        nc.scalar.dma_start(bqkT[:], bqkT_d[:])
        nc.scalar.dma_start(wvmat8[:], wvmat8_d[:])
        for t in range(HT):
            nc.scalar.dma_start(wqT8[:, t], wqT8_d[:, t])

        b0 = {(0, 8): eT8_00}
        for j in range(SC):
            if j:
                b0[j, 8] = encT_p.tile(
                    [128, HT, 512], FP8, tag="encT8", name=f"eT8_0{j}"
                )
            b0[j, 16] = enc16_p.tile(
                [128, HT, 512], BF16, tag="enc16", name=f"e16_0{j}"
            )
        nc.sync.dma_start(b0[1, 8][:], encT8_d[0, 1])
        nc.sync.dma_start(b0[2, 8][:], encT8_d[0, 2])
        nc.sync.dma_start(b0[0, 16][:], encT16_d[0, 0])
        nc.sync.dma_start(b0[3, 8][:], encT8_d[0, 3])
        nc.sync.dma_start(b0[1, 16][:], encT16_d[0, 1])
        nc.sync.dma_start(b0[2, 16][:], encT16_d[0, 2])
        nc.sync.dma_start(b0[3, 16][:], encT16_d[0, 3])
        staged = {(0, j): (b0[j, 8], b0[j, 16]) for j in range(SC)}

        qkb = consts.tile([128, HT, BPC], F32, tag="qkb")
        scr = consts.tile([128, 512], BF16, tag="scr")

        def kproj_mm_chain(i, eT8):
            pk = kp.tile([128, 512], F32, tag="kp", name="pk")
            for kt in range(KT):
                nc.tensor.matmul(
                    pk[:],
                    wkT8[:, i, ts(kt, 2), :],
                    eT8[:, ts(kt, 2), :],
                    start=(kt == 0),
                    stop=(kt == KT - 1),
                    perf_mode=DR,
                )
            return pk

        def qproj_t(t):
            # q^T + bq + bk, one o-tile: qkb[:, t, b] = q/512 + bq + bk
            pq = pq_p.tile([128, BPC], F32, tag="pq")
            for kt in range(KT):
                nc.tensor.matmul(
                    pq[:],
                    wqT8[:, t, ts(kt, 2), :],
                    hidT8[:, ts(kt, 2), :],
                    start=(kt == 0),
                    stop=(kt == KT - 1),
                    perf_mode=DR,
                )
            nc.vector.tensor_scalar(
                qkb[:, t, :], pq[:], 1.0 / (WK_SCALE * HID_SCALE),
                bqkT[:, t : t + 1], op0=Mult, op1=Add,
            )

        def scores_mm(c, u):
            # 64*scores chunk via fp8 DR: contraction over 2 o-chunks x 128
            # partitions at once; M=128 replicated rows = broadcast result.
            nc.tensor.matmul(
                c["ps"][:],
                wvmat8[:, u],
                c["eT_j"][:, ts(u, 2), :],
                start=(u == 0),
                stop=(u == HU - 1),
                perf_mode=DR,
            )

        def tail_exp(p):
            attn = sc_p.tile([128, 512], BF16, tag="attn")
            nc.scalar.activation(
                attn[:], p["ps"][:], Exp, scale=1.0 / WK_SCALE,
                accum_out=ssum4s[p["b"]][:, p["j"] : p["j"] + 1],
            )
            p["attn"] = attn

        def tail_einsum(p):
            # attn-weighted sum of enc rows on DVE: acc4[p_, i, j] =
            # sum_s attn[s] * enc[512j+s, 128i+p_].
            b, j = p["b"], p["j"]
            for i in range(HT):
                nc.vector.scalar_tensor_tensor(
                    scr[:], p["e16"][:, i, :], 1.0, p["attn"][:],
                    op0=Mult, op1=Mult,
                    accum_out=acc4s[b][:, i, j : j + 1],
                )

        def finish_batch(b):
            accH = batch.tile([128, HT], F32, tag="accH", bufs=2)
            nc.vector.reduce_sum(accH[:], acc4s[b][:], axis=X)
            ssum = batch.tile([128, 1], F32, tag="ssum", bufs=2)
            nc.vector.reduce_sum(ssum[:], ssum4s[b][:], axis=X)
            inv = batch.tile([128, 1], F32, tag="inv", bufs=2)
            nc.vector.reciprocal(inv[:], ssum[:])
            outF = batch.tile([128, HT], F32, tag="outF", bufs=2)
            nc.vector.tensor_scalar_mul(outF[:], accH[:], inv[:, 0:1])
            nc.sync.dma_start(outT_d[b][:], outF[:])

        # per-batch accumulators
        acc4s = [
            batch.tile([128, HT, SC], F32, tag=f"acc4_{b}", name=f"acc4_{b}")
            for b in range(BPC)
        ]
        ssum4s = [
            batch.tile([128, SC], F32, tag=f"ssum4_{b}", name=f"ssum4_{b}")
            for b in range(BPC)
        ]

        pending = None
        for b in range(BPC):
            for j in range(SC):
                # stage chunk (b+1, j) while computing chunk (b, j)
                if b + 1 < BPC:
                    staged[(b + 1, j)] = load_enc_chunk(b + 1, j)
                eT8, e16 = staged.pop((b, j))
                p = pending
                first = b == 0 and j == 0
                cur = dict(
                    b=b, j=j, e16=e16,
                    eT_j=eT_p.tile([128, HT, 512], FP8, tag="eTj", name="eTj"),
                    ps=ps_p.tile([128, 512], F32, tag="ps", name="ps"),
                )
                for i in range(HT):
                    pk = kproj_mm_chain(i, eT8)
                    if first:
                        qproj_t(i)
                    nc.scalar.activation(
                        cur["eT_j"][:, i, :], pk[:], Tanh,
                        bias=qkb[:, i, b : b + 1], scale=1.0 / WK_SCALE,
                    )
                    if i == 0 and p is not None:
                        # prev chunk's last scores MM + its tail
                        scores_mm(p, HU - 1)
                        tail_exp(p)
                        tail_einsum(p)
                        if p["j"] == SC - 1:
                            finish_batch(p["b"])
                    if i >= 3 and i % 2 == 1:
                        # scores DR pair (i-3, i-2) of THIS chunk: both tanhs
                        # are done by the time chain_i finishes streaming
                        scores_mm(cur, (i - 3) // 2)
                pending = cur

        # flush the final chunk
        p = pending
        scores_mm(p, HU - 1)
        tail_exp(p)
        tail_einsum(p)
        finish_batch(p["b"])

    nc.compile()
    return nc


_CACHED_NC = None


def _get_nc():
    global _CACHED_NC
    if _CACHED_NC is None:
        _CACHED_NC = build_program()
    return _CACHED_NC


_F8 = mybir.dt.np(FP8)
_BF = mybir.dt.np(BF16)


def make_in_maps(hidden, encoder_outputs, Wq, bq, Wk, bk, wv):
    """Host-side shard + layout prep (all compute FLOPs stay on device)."""
    hid_last = np.asarray(hidden, np.float32)[-1]  # [32, H]
    enc = np.asarray(encoder_outputs, np.float32)
    Wq = np.asarray(Wq, np.float32)
    Wk = np.asarray(Wk, np.float32)
    wv = np.asarray(wv, np.float32)
    bqkT = np.ascontiguousarray(
        (np.asarray(bq, np.float32) + np.asarray(bk, np.float32)).reshape(HT, 128).T
    )
    # rank-1 paired scores weights: [128(p), HU(u), 2(r), 128(m)] = 64*wv
    wvT = wv.reshape(HT, 128).T  # [128, HT]
    wvmat8 = np.ascontiguousarray(
        np.broadcast_to(
            (wvT * WK_SCALE).reshape(128, HU, 2)[:, :, :, None], (128, HU, 2, 128)
        )
    ).astype(_F8)

    # enc^T tiles: [B, SC, 128(p), HT(i), 512(s)]
    encT = np.ascontiguousarray(
        np.clip(enc, -240, 240)
        .reshape(B, SC, 512, HT, 128)
        .transpose(0, 1, 4, 3, 2)
    )
    encT8 = encT.astype(_F8)
    encT16 = encT.astype(_BF)

    # W^T fp8 with x64 prescale: [128(p), HT(i), HT(c), 128(m)]
    def wT8(W):
        return np.ascontiguousarray(
            np.clip(W * WK_SCALE, -240, 240)
            .reshape(HT, 128, HT, 128)
            .transpose(3, 0, 2, 1)
        ).astype(_F8)

    wkT8 = wT8(Wk)
    wqT8 = wT8(Wq)

    in_maps = []
    for c in range(NCORES):
        sl = slice(c * BPC, (c + 1) * BPC)
        # hid^T fp8 x8: [128(p), HT(c), BPC(b)]
        hidT8 = np.ascontiguousarray(
            np.clip(hid_last[sl] * HID_SCALE, -240, 240)
            .reshape(BPC, HT, 128)
            .transpose(2, 1, 0)
        ).astype(_F8)
        in_maps.append(
            {
                "encT8": np.ascontiguousarray(encT8[sl]),
                "encT16": np.ascontiguousarray(encT16[sl]),
                "wkT8": wkT8,
                "wqT8": wqT8,
                "hidT8": hidT8,
                "bqkT": bqkT,
                "wvmat8": wvmat8,
            }
        )
    return in_maps


def run(inputs, trace=False):
    """Run on hardware; returns (output [32,1,1024], BassKernelResults)."""
    nc = _get_nc()
    in_maps = make_in_maps(
        inputs["hidden"],
        inputs["encoder_outputs"],
        inputs["Wq"],
        inputs["bq"],
        inputs["Wk"],
        inputs["bk"],
        inputs["wv"],
    )
    res = run_bass_kernel_spmd(nc, in_maps, list(range(NCORES)), trace=trace)
    # outT[b, p, i] -> out[b, 128i+p]
    outs = []
    for c in range(NCORES):
        outT = res.results[c]["outT"]  # [BPC, 128, HT]
        outs.append(outT.transpose(0, 2, 1).reshape(BPC, 1, H))
    out = np.concatenate(outs, axis=0)
    return out.astype(np.float32), res


def kernel(hidden, encoder_outputs, Wq, bq, Wk, bk, wv, bv):
    out, _ = run(
        {
            "hidden": hidden,
            "encoder_outputs": encoder_outputs,
            "Wq": Wq,
            "bq": bq,
            "Wk": Wk,
            "bk": bk,
            "wv": wv,
        }
    )
    return out


# revision 21
# speedup vs baseline: 1.1253x; 1.0407x over previous
"""Bahdanau attention kernel for 8 Trainium2 NeuronCores.

Problem shapes (hardcoded): hidden [2, 32, 1024], encoder_outputs [32, 2048, 1024],
Wq/Wk [1024, 1024], bq/bk/wv [1024], bv scalar. Output [32, 1, 1024].

Sharding: data-parallel over batch B=32 -> 4 batches per core, weights replicated.
bv is dropped entirely (softmax is invariant to constant shifts).

Key structure (v8):
- The PE's main job is the K-projection (enc @ Wk.T) in fp8e4 DoubleRow
  (2 fp8 MACs/cell/cycle): 4 accumulating MMs of contraction 256 per
  (o-tile, s-chunk). Wk is pre-scaled x64 on the host; the inverse folds
  into the tanh scale. The q-projection also runs fp8 DR, interleaved into
  chunk (0,0)'s chains so it costs no serial ramp time.
- The tanh output is written in fp8e4 (its ONLY consumer is the scores
  contraction; the einsum reads the bf16 enc copy directly), so the scores
  wv-contraction also runs in fp8 DoubleRow: 4 chained MMs per chunk with
  RANK-1 paired weights (wvmat8[:, u, r, m] = 64*wv[128(2u+r)+p]), each
  contracting 2 o-chunks x 128 partitions at once. The [128, 512] PSUM
  result is 64*scores replicated on every partition - the einsum broadcast
  comes for free and the x64 folds into the exp's free scale. No transposes
  and no cross-engine scores traffic anywhere.
- exp on ACT straight from the scores PSUM with scale=1/64 (no max-shift
  needed: |scores| <= sum|wv| <= 32), accum_out -> softmax denominator.
- attn @ enc einsum: 8 scalar_tensor_tensor ops with accum_out on DVE over
  the transposed bf16 enc tiles (same [h-part, s] tiling as the fp8 tiles);
  per-chunk partials land in acc4[:, i, j]; one reduce at batch end.
- The final [128, 8] column is scaled by the reciprocal softmax sum and
  written transposed (outT); the host undoes the transpose.
- Ramp: the two DMA queues run in parallel (scalar: first fp8 enc chunk +
  per-tile wqT8 + smalls; sync: wkT8 halves + the enc rings).
"""

from contextlib import ExitStack

import numpy as np

import concourse.bacc as bacc
import concourse.bass as bass
import concourse.mybir as mybir
import concourse.tile as tile
from concourse.bass_utils import run_bass_kernel_spmd

B, S, H = 32, 2048, 1024
NCORES = 8
BPC = B // NCORES  # 4 batches per core
F32 = mybir.dt.float32
BF16 = mybir.dt.bfloat16
FP8 = mybir.dt.float8e4
HT = H // 128  # 8 chunks of 128 along h or o
HU = HT // 2  # 4 DoubleRow o-chunk pairs for the scores contraction
SC = S // 512  # 4 s-chunks of 512
KT = 4  # fp8 DoubleRow: 4 contraction steps of 256
WK_SCALE = 64.0
HID_SCALE = 8.0
Tanh = mybir.ActivationFunctionType.Tanh
Exp = mybir.ActivationFunctionType.Exp
X = mybir.AxisListType.X
DR = mybir.MatmulPerfMode.DoubleRow
Mult = mybir.AluOpType.mult
Add = mybir.AluOpType.add

ts = bass.ts


def build_program():
    nc = bacc.Bacc("TRN2", target_bir_lowering=False, debug=False)

    # enc^T fp8 tiles: encT8[b, j, p, i, s] = fp8(enc[b, 512j+s, 128i+p])
    encT8_d = nc.dram_tensor("encT8", [BPC, SC, 128, HT, 512], FP8, kind="ExternalInput")
    # enc^T bf16 tiles, same layout (einsum operand)
    encT16_d = nc.dram_tensor("encT16", [BPC, SC, 128, HT, 512], BF16, kind="ExternalInput")
    # Wk^T fp8 (x64): wkT8[p, i, c, m] = fp8(64 * Wk[128i+m, 128c+p])
    wkT8_d = nc.dram_tensor("wkT8", [128, HT, HT, 128], FP8, kind="ExternalInput")
    # Wq^T fp8 (x64): wqT8[p, t, c, n] = fp8(64 * Wq[128t+n, 128c+p])
    wqT8_d = nc.dram_tensor("wqT8", [128, HT, HT, 128], FP8, kind="ExternalInput")
    # hid^T fp8 (x8): hidT8[p, c, b] = fp8(8 * hidden[-1][b, 128c+p])
    hidT8_d = nc.dram_tensor("hidT8", [128, HT, BPC], FP8, kind="ExternalInput")
    bqkT_d = nc.dram_tensor("bqkT", [128, HT], F32, kind="ExternalInput")  # (bq+bk)^T
    # rank-1 paired scores weights (x64): wvmat8[p, u, r, m] = fp8(64*wv[128(2u+r)+p])
    wvmat8_d = nc.dram_tensor("wvmat8", [128, HU, 2, 128], FP8, kind="ExternalInput")
    # transposed output: outT[b, p, i] = out[b, 128i+p]
    outT_d = nc.dram_tensor("outT", [BPC, 128, HT], F32, kind="ExternalOutput")

    with tile.TileContext(nc) as tc, ExitStack() as ctx:
        consts = ctx.enter_context(tc.tile_pool(name="consts", bufs=1))
        kp = ctx.enter_context(tc.tile_pool(name="kp", bufs=5, space="PSUM"))
        ps_p = ctx.enter_context(tc.tile_pool(name="psp", bufs=2, space="PSUM"))
        pq_p = ctx.enter_context(tc.tile_pool(name="pqp", bufs=1, space="PSUM"))
        encT_p = ctx.enter_context(tc.tile_pool(name="encT", bufs=5))  # 512KB/slot
        enc16_p = ctx.enter_context(tc.tile_pool(name="enc16", bufs=6))  # 1MB/slot
        eT_p = ctx.enter_context(tc.tile_pool(name="eT", bufs=2))  # 512KB/slot
        sc_p = ctx.enter_context(tc.tile_pool(name="sc", bufs=2))
        batch = ctx.enter_context(tc.tile_pool(name="batch", bufs=1))

        # ---- staging helpers (chunk granular DMAs; the fp8 kproj operand
        # rides the sync queue, the bf16 einsum operand rides the otherwise
        # idle gpsimd queue so neither stream blocks the other) ----
        def load_enc_chunk(b, j, e16_q=None):
            eT8 = encT_p.tile([128, HT, 512], FP8, tag="encT8")
            nc.sync.dma_start(eT8[:], encT8_d[b, j])
            e16 = enc16_p.tile([128, HT, 512], BF16, tag="enc16")
            (e16_q or nc.sync).dma_start(e16[:], encT16_d[b, j])
            return eT8, e16

        # ---- weights + consts. Critical path to the first kproj chain:
        # scalar queue carries the first fp8 enc chunk; sync queue carries
        # wkT8's lower half. Everything else lands behind those. ----
        wkT8 = consts.tile([128, HT, HT, 128], FP8, tag="wkT8")
        eT8_00 = encT_p.tile([128, HT, 512], FP8, tag="encT8", name="eT8_00")
        hidT8 = consts.tile([128, HT, BPC], FP8, tag="hidT8")
        bqkT = consts.tile([128, HT], F32, tag="bqkT")
        wqT8 = consts.tile([128, HT, HT, 128], FP8, tag="wqT8")
        wvmat8 = consts.tile([128, HU, 2, 128], FP8, tag="wvmat8")

        # Ramp: scalar queue carries the first fp8 enc chunk + qproj inputs;
        # the sync queue carries wkT8 halves then batch 0's chunks strictly
        # in need-time order (the bf16 einsum copies are consumed one chunk
        # later than the fp8 tiles, so they interleave behind them).
        nc.sync.dma_start(wkT8[:, 0:4], wkT8_d[:, 0:4])
        nc.scalar.dma_start(eT8_00[:], encT8_d[0, 0])
        nc.sync.dma_start(wkT8[:, 4:8], wkT8_d[:, 4:8])
        nc.scalar.dma_start(hidT8[:], hidT8_d[:])
        nc.scalar.dma_start(bqkT[:], bqkT_d[:])
        nc.scalar.dma_start(wvmat8[:], wvmat8_d[:])
        for t in range(HT):
            nc.scalar.dma_start(wqT8[:, t], wqT8_d[:, t])

        # batch 0: fp8 tiles in need-order on sync; the bf16 einsum copies
        # ride the scalar queue (idle once the ramp consts are in), giving
        # two parallel staging streams while the DMA rate is still cold.
        b0 = {(0, 8): eT8_00}
        for j in range(SC):
            if j:
                b0[j, 8] = encT_p.tile(
                    [128, HT, 512], FP8, tag="encT8", name=f"eT8_0{j}"
                )
            b0[j, 16] = enc16_p.tile(
                [128, HT, 512], BF16, tag="enc16", name=f"e16_0{j}"
            )
        nc.sync.dma_start(b0[1, 8][:], encT8_d[0, 1])
        nc.sync.dma_start(b0[2, 8][:], encT8_d[0, 2])
        nc.sync.dma_start(b0[3, 8][:], encT8_d[0, 3])
        for j in range(SC):
            nc.scalar.dma_start(b0[j, 16][:], encT16_d[0, j])
        staged = {(0, j): (b0[j, 8], b0[j, 16]) for j in range(SC)}

        qkb = consts.tile([128, HT, BPC], F32, tag="qkb")
        scr = consts.tile([128, 512], BF16, tag="scr")

        def kproj_mm_chain(i, eT8):
            pk = kp.tile([128, 512], F32, tag="kp", name="pk")
            for kt in range(KT):
                nc.tensor.matmul(
                    pk[:],
                    wkT8[:, i, ts(kt, 2), :],
                    eT8[:, ts(kt, 2), :],
                    start=(kt == 0),
                    stop=(kt == KT - 1),
                    perf_mode=DR,
                )
            return pk

        def qproj_t(t):
            # q^T + bq + bk, one o-tile: qkb[:, t, b] = q/512 + bq + bk
            pq = pq_p.tile([128, BPC], F32, tag="pq")
            for kt in range(KT):
                nc.tensor.matmul(
                    pq[:],
                    wqT8[:, t, ts(kt, 2), :],
                    hidT8[:, ts(kt, 2), :],
                    start=(kt == 0),
                    stop=(kt == KT - 1),
                    perf_mode=DR,
                )
            nc.vector.tensor_scalar(
                qkb[:, t, :], pq[:], 1.0 / (WK_SCALE * HID_SCALE),
                bqkT[:, t : t + 1], op0=Mult, op1=Add,
            )

        def scores_mm(c, u):
            # 64*scores chunk via fp8 DR: contraction over 2 o-chunks x 128
            # partitions at once; M=128 replicated rows = broadcast result.
            nc.tensor.matmul(
                c["ps"][:],
                wvmat8[:, u],
                c["eT_j"][:, ts(u, 2), :],
                start=(u == 0),
                stop=(u == HU - 1),
                perf_mode=DR,
            )

        def tail_exp(p):
            attn = sc_p.tile([128, 512], BF16, tag="attn")
            nc.scalar.activation(
                attn[:], p["ps"][:], Exp, scale=1.0 / WK_SCALE,
                accum_out=ssum4s[p["b"]][:, p["j"] : p["j"] + 1],
            )
            p["attn"] = attn

        def tail_einsum(p):
            # attn-weighted sum of enc rows on DVE: acc4[p_, i, j] =
            # sum_s attn[s] * enc[512j+s, 128i+p_].
            b, j = p["b"], p["j"]
            for i in range(HT):
                nc.vector.scalar_tensor_tensor(
                    scr[:], p["e16"][:, i, :], 1.0, p["attn"][:],
                    op0=Mult, op1=Mult,
                    accum_out=acc4s[b][:, i, j : j + 1],
                )

        def finish_batch(b):
            accH = batch.tile([128, HT], F32, tag="accH", bufs=2)
            nc.vector.reduce_sum(accH[:], acc4s[b][:], axis=X)
            ssum = batch.tile([128, 1], F32, tag="ssum", bufs=2)
            nc.vector.reduce_sum(ssum[:], ssum4s[b][:], axis=X)
            inv = batch.tile([128, 1], F32, tag="inv", bufs=2)
            nc.vector.reciprocal(inv[:], ssum[:])
            outF = batch.tile([128, HT], F32, tag="outF", bufs=2)
            nc.vector.tensor_scalar_mul(outF[:], accH[:], inv[:, 0:1])
            nc.sync.dma_start(outT_d[b][:], outF[:])

        # per-batch accumulators
        acc4s = [
            batch.tile([128, HT, SC], F32, tag=f"acc4_{b}", name=f"acc4_{b}")
            for b in range(BPC)
        ]
        ssum4s = [
            batch.tile([128, SC], F32, tag=f"ssum4_{b}", name=f"ssum4_{b}")
            for b in range(BPC)
        ]

        pending = None
        for b in range(BPC):
            for j in range(SC):
                # stage chunk (b+1, j) while computing chunk (b, j); batch
                # 1's bf16 copies still ride the scalar queue (the sync
                # queue is busy catching up while the DMA rate warms up)
                if b + 1 < BPC:
                    staged[(b + 1, j)] = load_enc_chunk(
                        b + 1, j, e16_q=nc.scalar if b == 0 else None
                    )
                eT8, e16 = staged.pop((b, j))
                p = pending
                first = b == 0 and j == 0
                cur = dict(
                    b=b, j=j, e16=e16,
                    eT_j=eT_p.tile([128, HT, 512], FP8, tag="eTj", name="eTj"),
                    ps=ps_p.tile([128, 512], F32, tag="ps", name="ps"),
                )
                for i in range(HT):
                    pk = kproj_mm_chain(i, eT8)
                    if first:
                        qproj_t(i)
                    nc.scalar.activation(
                        cur["eT_j"][:, i, :], pk[:], Tanh,
                        bias=qkb[:, i, b : b + 1], scale=1.0 / WK_SCALE,
                    )
                    if i == 0 and p is not None:
                        # prev chunk's last scores MM + its tail
                        scores_mm(p, HU - 1)
                        tail_exp(p)
                        tail_einsum(p)
                        if p["j"] == SC - 1:
                            finish_batch(p["b"])
                    if i >= 3 and i % 2 == 1:
                        # scores DR pair (i-3, i-2) of THIS chunk: both tanhs
                        # are done by the time chain_i finishes streaming
                        scores_mm(cur, (i - 3) // 2)
                pending = cur

        # flush the final chunk
        p = pending
        scores_mm(p, HU - 1)
        tail_exp(p)
        tail_einsum(p)
        finish_batch(p["b"])

    nc.compile()
    return nc


_CACHED_NC = None


def _get_nc():
    global _CACHED_NC
    if _CACHED_NC is None:
        _CACHED_NC = build_program()
    return _CACHED_NC


_F8 = mybir.dt.np(FP8)
_BF = mybir.dt.np(BF16)


def make_in_maps(hidden, encoder_outputs, Wq, bq, Wk, bk, wv):
    """Host-side shard + layout prep (all compute FLOPs stay on device)."""
    hid_last = np.asarray(hidden, np.float32)[-1]  # [32, H]
    enc = np.asarray(encoder_outputs, np.float32)
    Wq = np.asarray(Wq, np.float32)
    Wk = np.asarray(Wk, np.float32)
    wv = np.asarray(wv, np.float32)
    bqkT = np.ascontiguousarray(
        (np.asarray(bq, np.float32) + np.asarray(bk, np.float32)).reshape(HT, 128).T
    )
    # rank-1 paired scores weights: [128(p), HU(u), 2(r), 128(m)] = 64*wv
    wvT = wv.reshape(HT, 128).T  # [128, HT]
    wvmat8 = np.ascontiguousarray(
        np.broadcast_to(
            (wvT * WK_SCALE).reshape(128, HU, 2)[:, :, :, None], (128, HU, 2, 128)
        )
    ).astype(_F8)

    # enc^T tiles: [B, SC, 128(p), HT(i), 512(s)]
    encT = np.ascontiguousarray(
        np.clip(enc, -240, 240)
        .reshape(B, SC, 512, HT, 128)
        .transpose(0, 1, 4, 3, 2)
    )
    encT8 = encT.astype(_F8)
    encT16 = encT.astype(_BF)

    # W^T fp8 with x64 prescale: [128(p), HT(i), HT(c), 128(m)]
    def wT8(W):
        return np.ascontiguousarray(
            np.clip(W * WK_SCALE, -240, 240)
            .reshape(HT, 128, HT, 128)
            .transpose(3, 0, 2, 1)
        ).astype(_F8)

    wkT8 = wT8(Wk)
    wqT8 = wT8(Wq)

    in_maps = []
    for c in range(NCORES):
        sl = slice(c * BPC, (c + 1) * BPC)
        # hid^T fp8 x8: [128(p), HT(c), BPC(b)]
        hidT8 = np.ascontiguousarray(
            np.clip(hid_last[sl] * HID_SCALE, -240, 240)
            .reshape(BPC, HT, 128)
            .transpose(2, 1, 0)
        ).astype(_F8)
        in_maps.append(
            {
                "encT8": np.ascontiguousarray(encT8[sl]),
                "encT16": np.ascontiguousarray(encT16[sl]),
                "wkT8": wkT8,
                "wqT8": wqT8,
                "hidT8": hidT8,
                "bqkT": bqkT,
                "wvmat8": wvmat8,
            }
        )
    return in_maps


def run(inputs, trace=False):
    """Run on hardware; returns (output [32,1,1024], BassKernelResults)."""
    nc = _get_nc()
    in_maps = make_in_maps(
        inputs["hidden"],
        inputs["encoder_outputs"],
        inputs["Wq"],
        inputs["bq"],
        inputs["Wk"],
        inputs["bk"],
        inputs["wv"],
    )
    res = run_bass_kernel_spmd(nc, in_maps, list(range(NCORES)), trace=trace)
    # outT[b, p, i] -> out[b, 128i+p]
    outs = []
    for c in range(NCORES):
        outT = res.results[c]["outT"]  # [BPC, 128, HT]
        outs.append(outT.transpose(0, 2, 1).reshape(BPC, 1, H))
    out = np.concatenate(outs, axis=0)
    return out.astype(np.float32), res


def kernel(hidden, encoder_outputs, Wq, bq, Wk, bk, wv, bv):
    out, _ = run(
        {
            "hidden": hidden,
            "encoder_outputs": encoder_outputs,
            "Wq": Wq,
            "bq": bq,
            "Wk": Wk,
            "bk": bk,
            "wv": wv,
        }
    )
    return out


# revision 27
# speedup vs baseline: 1.1416x; 1.0145x over previous
"""Bahdanau attention kernel for 8 Trainium2 NeuronCores.

Problem shapes (hardcoded): hidden [2, 32, 1024], encoder_outputs [32, 2048, 1024],
Wq/Wk [1024, 1024], bq/bk/wv [1024], bv scalar. Output [32, 1, 1024].

Sharding: data-parallel over batch B=32 -> 4 batches per core, weights replicated.
bv is dropped entirely (softmax is invariant to constant shifts).

Key structure (v8):
- The PE's main job is the K-projection (enc @ Wk.T) in fp8e4 DoubleRow
  (2 fp8 MACs/cell/cycle): 4 accumulating MMs of contraction 256 per
  (o-tile, s-chunk). Wk is pre-scaled x64 on the host; the inverse folds
  into the tanh scale. The q-projection also runs fp8 DR, interleaved into
  chunk (0,0)'s chains so it costs no serial ramp time.
- The tanh output is written in fp8e4 (its ONLY consumer is the scores
  contraction; the einsum reads the bf16 enc copy directly), so the scores
  wv-contraction also runs in fp8 DoubleRow: 4 chained MMs per chunk with
  RANK-1 paired weights (wvmat8[:, u, r, m] = 64*wv[128(2u+r)+p]), each
  contracting 2 o-chunks x 128 partitions at once. The [128, 512] PSUM
  result is 64*scores replicated on every partition - the einsum broadcast
  comes for free and the x64 folds into the exp's free scale. No transposes
  and no cross-engine scores traffic anywhere.
- exp on ACT straight from the scores PSUM with scale=1/64 (no max-shift
  needed: |scores| <= sum|wv| <= 32), accum_out -> softmax denominator.
- attn @ enc einsum: 8 scalar_tensor_tensor ops with accum_out on DVE over
  the transposed bf16 enc tiles (same [h-part, s] tiling as the fp8 tiles);
  per-chunk partials land in acc4[:, i, j]; one reduce at batch end.
- The final [128, 8] column is scaled by the reciprocal softmax sum and
  written transposed (outT); the host undoes the transpose.
- Ramp: the two DMA queues run in parallel (scalar: first fp8 enc chunk +
  per-tile wqT8 + smalls; sync: wkT8 halves + the enc rings).
"""

from contextlib import ExitStack

import numpy as np

import concourse.bacc as bacc
import concourse.bass as bass
import concourse.mybir as mybir
import concourse.tile as tile
from concourse.bass_utils import run_bass_kernel_spmd

B, S, H = 32, 2048, 1024
NCORES = 8
BPC = B // NCORES  # 4 batches per core
F32 = mybir.dt.float32
BF16 = mybir.dt.bfloat16
FP8 = mybir.dt.float8e4
HT = H // 128  # 8 chunks of 128 along h or o
HU = HT // 2  # 4 DoubleRow o-chunk pairs for the scores contraction
SC = S // 512  # 4 s-chunks of 512
KT = 4  # fp8 DoubleRow: 4 contraction steps of 256
WK_SCALE = 64.0
HID_SCALE = 8.0
Tanh = mybir.ActivationFunctionType.Tanh
Exp = mybir.ActivationFunctionType.Exp
X = mybir.AxisListType.X
DR = mybir.MatmulPerfMode.DoubleRow
Mult = mybir.AluOpType.mult
Add = mybir.AluOpType.add

ts = bass.ts


def build_program():
    nc = bacc.Bacc("TRN2", target_bir_lowering=False, debug=False)

    # enc^T fp8 tiles: encT8[b, j, p, i, s] = fp8(enc[b, 512j+s, 128i+p])
    encT8_d = nc.dram_tensor("encT8", [BPC, SC, 128, HT, 512], FP8, kind="ExternalInput")
    # enc^T bf16 tiles, same layout (einsum operand)
    encT16_d = nc.dram_tensor("encT16", [BPC, SC, 128, HT, 512], BF16, kind="ExternalInput")
    # Wk^T fp8 (x64): wkT8[p, i, c, m] = fp8(64 * Wk[128i+m, 128c+p])
    wkT8_d = nc.dram_tensor("wkT8", [128, HT, HT, 128], FP8, kind="ExternalInput")
    # Wq^T fp8 (x64): wqT8[p, t, c, n] = fp8(64 * Wq[128t+n, 128c+p])
    wqT8_d = nc.dram_tensor("wqT8", [128, HT, HT, 128], FP8, kind="ExternalInput")
    # hid^T fp8 (x8): hidT8[p, c, b] = fp8(8 * hidden[-1][b, 128c+p])
    hidT8_d = nc.dram_tensor("hidT8", [128, HT, BPC], FP8, kind="ExternalInput")
    bqkT_d = nc.dram_tensor("bqkT", [128, HT], F32, kind="ExternalInput")  # (bq+bk)^T
    # rank-1 paired scores weights (x64): wvmat8[p, u, r, m] = fp8(64*wv[128(2u+r)+p])
    wvmat8_d = nc.dram_tensor("wvmat8", [128, HU, 2, 128], FP8, kind="ExternalInput")
    # transposed output: outT[b, p, i] = out[b, 128i+p]
    outT_d = nc.dram_tensor("outT", [BPC, 128, HT], F32, kind="ExternalOutput")

    with tile.TileContext(nc) as tc, ExitStack() as ctx:
        consts = ctx.enter_context(tc.tile_pool(name="consts", bufs=1))
        kp = ctx.enter_context(tc.tile_pool(name="kp", bufs=5, space="PSUM"))
        ps_p = ctx.enter_context(tc.tile_pool(name="psp", bufs=2, space="PSUM"))
        pq_p = ctx.enter_context(tc.tile_pool(name="pqp", bufs=1, space="PSUM"))
        encT_p = ctx.enter_context(tc.tile_pool(name="encT", bufs=5))  # 512KB/slot
        enc16_p = ctx.enter_context(tc.tile_pool(name="enc16", bufs=6))  # 1MB/slot
        eT_p = ctx.enter_context(tc.tile_pool(name="eT", bufs=2))  # 512KB/slot
        sc_p = ctx.enter_context(tc.tile_pool(name="sc", bufs=2))
        batch = ctx.enter_context(tc.tile_pool(name="batch", bufs=1))

        # ---- staging helpers (chunk granular DMAs; the fp8 kproj operand
        # rides the sync queue, the bf16 einsum operand rides the otherwise
        # idle gpsimd queue so neither stream blocks the other) ----
        def load_enc_chunk(b, j, e16_q=None):
            eT8 = encT_p.tile([128, HT, 512], FP8, tag="encT8")
            nc.sync.dma_start(eT8[:], encT8_d[b, j])
            e16 = enc16_p.tile([128, HT, 512], BF16, tag="enc16")
            (e16_q or nc.sync).dma_start(e16[:], encT16_d[b, j])
            return eT8, e16

        # ---- weights + consts. Critical path to the first kproj chain:
        # scalar queue carries the first fp8 enc chunk; sync queue carries
        # wkT8's lower half. Everything else lands behind those. ----
        wkT8 = consts.tile([128, HT, HT, 128], FP8, tag="wkT8")
        eT8_00 = encT_p.tile([128, HT, 512], FP8, tag="encT8", name="eT8_00")
        hidT8 = consts.tile([128, HT, BPC], FP8, tag="hidT8")
        bqkT = consts.tile([128, HT], F32, tag="bqkT")
        wqT8 = consts.tile([128, HT, HT, 128], FP8, tag="wqT8")
        wvmat8 = consts.tile([128, HU, 2, 128], FP8, tag="wvmat8")

        # Ramp: scalar queue carries the first fp8 enc chunk + qproj inputs;
        # the sync queue carries wkT8 halves then batch 0's chunks strictly
        # in need-time order (the bf16 einsum copies are consumed one chunk
        # later than the fp8 tiles, so they interleave behind them).
        # the q-projection's small inputs ride the gpsimd queue so qproj
        # can start ~4us before wkT8/enc land on the other two queues
        nc.gpsimd.dma_start(hidT8[:], hidT8_d[:])
        nc.gpsimd.dma_start(bqkT[:], bqkT_d[:])
        nc.gpsimd.dma_start(wqT8[:, 0], wqT8_d[:, 0])
        nc.gpsimd.dma_start(wqT8[:, 1], wqT8_d[:, 1])
        nc.sync.dma_start(wkT8[:, 0], wkT8_d[:, 0])
        nc.scalar.dma_start(eT8_00[:], encT8_d[0, 0])
        nc.sync.dma_start(wkT8[:, 1:4], wkT8_d[:, 1:4])
        nc.sync.dma_start(wkT8[:, 4:8], wkT8_d[:, 4:8])
        for t in range(2, HT):
            nc.scalar.dma_start(wqT8[:, t], wqT8_d[:, t])
        nc.scalar.dma_start(wvmat8[:], wvmat8_d[:])

        # batch 0: fp8 tiles in need-order on sync; the bf16 einsum copies
        # ride the scalar queue (idle once the ramp consts are in), giving
        # two parallel staging streams while the DMA rate is still cold.
        b0 = {(0, 8): eT8_00}
        for j in range(SC):
            if j:
                b0[j, 8] = encT_p.tile(
                    [128, HT, 512], FP8, tag="encT8", name=f"eT8_0{j}"
                )
            b0[j, 16] = enc16_p.tile(
                [128, HT, 512], BF16, tag="enc16", name=f"e16_0{j}"
            )
        nc.sync.dma_start(b0[1, 8][:], encT8_d[0, 1])
        nc.sync.dma_start(b0[2, 8][:], encT8_d[0, 2])
        nc.sync.dma_start(b0[3, 8][:], encT8_d[0, 3])
        for j in range(SC):
            nc.scalar.dma_start(b0[j, 16][:], encT16_d[0, j])
        staged = {(0, j): (b0[j, 8], b0[j, 16]) for j in range(SC)}

        qkb = consts.tile([128, HT, BPC], F32, tag="qkb")
        scr = consts.tile([128, 512], BF16, tag="scr")
        scr2 = consts.tile([128, 512], BF16, tag="scr2")

        def kproj_mm_chain(i, eT8):
            pk = kp.tile([128, 512], F32, tag="kp", name="pk")
            for kt in range(KT):
                nc.tensor.matmul(
                    pk[:],
                    wkT8[:, i, ts(kt, 2), :],
                    eT8[:, ts(kt, 2), :],
                    start=(kt == 0),
                    stop=(kt == KT - 1),
                    perf_mode=DR,
                )
            return pk

        def qproj_t(t):
            # q^T + bq + bk, one o-tile: qkb[:, t, b] = q/512 + bq + bk
            pq = pq_p.tile([128, BPC], F32, tag="pq")
            for kt in range(KT):
                nc.tensor.matmul(
                    pq[:],
                    wqT8[:, t, ts(kt, 2), :],
                    hidT8[:, ts(kt, 2), :],
                    start=(kt == 0),
                    stop=(kt == KT - 1),
                    perf_mode=DR,
                )
            nc.vector.tensor_scalar(
                qkb[:, t, :], pq[:], 1.0 / (WK_SCALE * HID_SCALE),
                bqkT[:, t : t + 1], op0=Mult, op1=Add,
            )

        def scores_mm(c, u):
            # 64*scores chunk via fp8 DR: contraction over 2 o-chunks x 128
            # partitions at once; M=128 replicated rows = broadcast result.
            nc.tensor.matmul(
                c["ps"][:],
                wvmat8[:, u],
                c["eT_j"][:, ts(u, 2), :],
                start=(u == 0),
                stop=(u == HU - 1),
                perf_mode=DR,
            )

        def tail_exp(p):
            attn = sc_p.tile([128, 512], BF16, tag="attn")
            nc.scalar.activation(
                attn[:], p["ps"][:], Exp, scale=1.0 / WK_SCALE,
                accum_out=ssum4s[p["b"]][:, p["j"] : p["j"] + 1],
            )
            p["attn"] = attn

        def tail_einsum(p):
            # attn-weighted sum of enc rows on DVE: acc4[p_, i, j] =
            # sum_s attn[s] * enc[512j+s, 128i+p_]. For the last two chunks
            # the upper o-chunks run on the otherwise-idle GPSIMD engine so
            # the serial einsum doesn't tail the whole kernel.
            b, j = p["b"], p["j"]
            for i in range(HT):
                nc.vector.scalar_tensor_tensor(
                    scr[:], p["e16"][:, i, :], 1.0, p["attn"][:],
                    op0=Mult, op1=Mult,
                    accum_out=acc4s[b][:, i, j : j + 1],
                )

        def finish_batch(b):
            accH = batch.tile([128, HT], F32, tag="accH", bufs=2)
            nc.vector.reduce_sum(accH[:], acc4s[b][:], axis=X)
            ssum = batch.tile([128, 1], F32, tag="ssum", bufs=2)
            nc.vector.reduce_sum(ssum[:], ssum4s[b][:], axis=X)
            inv = batch.tile([128, 1], F32, tag="inv", bufs=2)
            nc.vector.reciprocal(inv[:], ssum[:])
            outF = batch.tile([128, HT], F32, tag="outF", bufs=2)
            nc.vector.tensor_scalar_mul(outF[:], accH[:], inv[:, 0:1])
            nc.sync.dma_start(outT_d[b][:], outF[:])

        # per-batch accumulators
        acc4s = [
            batch.tile([128, HT, SC], F32, tag=f"acc4_{b}", name=f"acc4_{b}")
            for b in range(BPC)
        ]
        ssum4s = [
            batch.tile([128, SC], F32, tag=f"ssum4_{b}", name=f"ssum4_{b}")
            for b in range(BPC)
        ]

        pending = None
        for b in range(BPC):
            for j in range(SC):
                # stage chunk (b+1, j) while computing chunk (b, j); batch
                # 1's bf16 copies still ride the scalar queue (the sync
                # queue is busy catching up while the DMA rate warms up)
                if b + 1 < BPC:
                    staged[(b + 1, j)] = load_enc_chunk(
                        b + 1, j, e16_q=nc.scalar if b == 0 else None
                    )
                eT8, e16 = staged.pop((b, j))
                p = pending
                first = b == 0 and j == 0
                cur = dict(
                    b=b, j=j, e16=e16,
                    eT_j=eT_p.tile([128, HT, 512], FP8, tag="eTj", name="eTj"),
                    ps=ps_p.tile([128, 512], F32, tag="ps", name="ps"),
                )
                for i in range(HT):
                    if first:
                        qproj_t(i)
                    pk = kproj_mm_chain(i, eT8)
                    nc.scalar.activation(
                        cur["eT_j"][:, i, :], pk[:], Tanh,
                        bias=qkb[:, i, b : b + 1], scale=1.0 / WK_SCALE,
                    )
                    if i == 0 and p is not None:
                        # prev chunk's last scores MM + its tail
                        scores_mm(p, HU - 1)
                        tail_exp(p)
                        tail_einsum(p)
                        if p["j"] == SC - 1:
                            finish_batch(p["b"])
                    if i >= 3 and i % 2 == 1:
                        # scores DR pair (i-3, i-2) of THIS chunk: both tanhs
                        # are done by the time chain_i finishes streaming
                        scores_mm(cur, (i - 3) // 2)
                pending = cur

        # flush the final chunk
        p = pending
        scores_mm(p, HU - 1)
        tail_exp(p)
        tail_einsum(p)
        finish_batch(p["b"])

    nc.compile()
    return nc


_CACHED_NC = None


def _get_nc():
    global _CACHED_NC
    if _CACHED_NC is None:
        _CACHED_NC = build_program()
    return _CACHED_NC


_F8 = mybir.dt.np(FP8)
_BF = mybir.dt.np(BF16)


def make_in_maps(hidden, encoder_outputs, Wq, bq, Wk, bk, wv):
    """Host-side shard + layout prep (all compute FLOPs stay on device)."""
    hid_last = np.asarray(hidden, np.float32)[-1]  # [32, H]
    enc = np.asarray(encoder_outputs, np.float32)
    Wq = np.asarray(Wq, np.float32)
    Wk = np.asarray(Wk, np.float32)
    wv = np.asarray(wv, np.float32)
    bqkT = np.ascontiguousarray(
        (np.asarray(bq, np.float32) + np.asarray(bk, np.float32)).reshape(HT, 128).T
    )
    # rank-1 paired scores weights: [128(p), HU(u), 2(r), 128(m)] = 64*wv
    wvT = wv.reshape(HT, 128).T  # [128, HT]
    wvmat8 = np.ascontiguousarray(
        np.broadcast_to(
            (wvT * WK_SCALE).reshape(128, HU, 2)[:, :, :, None], (128, HU, 2, 128)
        )
    ).astype(_F8)

    # enc^T tiles: [B, SC, 128(p), HT(i), 512(s)]
    encT = np.ascontiguousarray(
        np.clip(enc, -240, 240)
        .reshape(B, SC, 512, HT, 128)
        .transpose(0, 1, 4, 3, 2)
    )
    encT8 = encT.astype(_F8)
    encT16 = encT.astype(_BF)

    # W^T fp8 with x64 prescale: [128(p), HT(i), HT(c), 128(m)]
    def wT8(W):
        return np.ascontiguousarray(
            np.clip(W * WK_SCALE, -240, 240)
            .reshape(HT, 128, HT, 128)
            .transpose(3, 0, 2, 1)
        ).astype(_F8)

    wkT8 = wT8(Wk)
    wqT8 = wT8(Wq)

    in_maps = []
    for c in range(NCORES):
        sl = slice(c * BPC, (c + 1) * BPC)
        # hid^T fp8 x8: [128(p), HT(c), BPC(b)]
        hidT8 = np.ascontiguousarray(
            np.clip(hid_last[sl] * HID_SCALE, -240, 240)
            .reshape(BPC, HT, 128)
            .transpose(2, 1, 0)
        ).astype(_F8)
        in_maps.append(
            {
                "encT8": np.ascontiguousarray(encT8[sl]),
                "encT16": np.ascontiguousarray(encT16[sl]),
                "wkT8": wkT8,
                "wqT8": wqT8,
                "hidT8": hidT8,
                "bqkT": bqkT,
                "wvmat8": wvmat8,
            }
        )
    return in_maps


def run(inputs, trace=False):
    """Run on hardware; returns (output [32,1,1024], BassKernelResults)."""
    nc = _get_nc()
    in_maps = make_in_maps(
        inputs["hidden"],
        inputs["encoder_outputs"],
        inputs["Wq"],
        inputs["bq"],
        inputs["Wk"],
        inputs["bk"],
        inputs["wv"],
    )
    res = run_bass_kernel_spmd(nc, in_maps, list(range(NCORES)), trace=trace)
    # outT[b, p, i] -> out[b, 128i+p]
    outs = []
    for c in range(NCORES):
        outT = res.results[c]["outT"]  # [BPC, 128, HT]
        outs.append(outT.transpose(0, 2, 1).reshape(BPC, 1, H))
    out = np.concatenate(outs, axis=0)
    return out.astype(np.float32), res


def kernel(hidden, encoder_outputs, Wq, bq, Wk, bk, wv, bv):
    out, _ = run(
        {
            "hidden": hidden,
            "encoder_outputs": encoder_outputs,
            "Wq": Wq,
            "bq": bq,
            "Wk": Wk,
            "bk": bk,
            "wv": wv,
        }
    )
    return out
